# revision 6
# baseline (speedup 1.0000x reference)
"""TRN2 Bass kernel for nn_CrossAttentionHeightSplit (v2).

26-view cross-attention, 2 scenes. For each (scene b, view i): q = x[b,i]
(1024 tokens, C=256), kv = concat of x[b, sel(i)] (3-4 neighbor views),
8-head MHA with weight group mha_index(i).

Sharding: 52 view-tasks over 8 cores. Each core runs an identical (SPMD)
program of 7 slots: 4 full 4-neighbor views, 2 full 3-neighbor views and
one 4-neighbor half-view (q=512). Full-view slots project K/V once for
both query halves (the v1 baseline projected them twice).

Per-slot dataflow (channel-major [c, tokens] layouts, f32r projections,
bf16 attention):
  qpT = WqT.T @ xq       [256, Q]  f32r matmuls -> bf16
  kpT = WkT.T @ xn       [256, n*1024] bf16
  v   = xn.T @ WvT       [kv, 8 heads x 33] bf16; 33rd col = ones so the
                         AV matmul also yields the softmax denominator
  scores per (qhalf, head, kv-chunk pair): PE K=32 slices, 4-way
        tile_position packing; PSUM [128,1024] tiles evacuated by EITHER
        - ACT: exp(scale*s) directly PSUM -> bf16 SBUF, or
        - DVE: Schraudolph fast-exp int16 = round(A*s + B), bitcast bf16
        so both engines drain PSUM in parallel (this is the bottleneck).
  av[33, Q] accumulated in PSUM over kv chunks (2-way col-group packing),
  normalize: DVE reciprocal + GPSIMD partition-broadcast + DVE mul,
  oT = WoT.T @ avnT      [256, Q] f32r -> DMA out.

Biases are zero for this problem (spec fill=zeros); a bias-capable
variant is compiled only if nonzero biases ever show up.
"""

import sys
import numpy as np

try:
    import concourse.bass as bass  # noqa: F401
except ImportError:
    sys.path.insert(0, "/opt/trn_rl_repo")

import concourse.bacc as bacc
import concourse.mybir as mybir
import concourse.tile as tile
from concourse.bass_utils import run_bass_kernel_spmd

dt = mybir.dt
AF = mybir.ActivationFunctionType
AL = mybir.AluOpType

# ---------------------------------------------------------------- constants
N_VIEWS = 26
C = 256
S = 1024          # tokens per view
NH = 8            # heads
D = 32            # head dim
ISQ = float(1.0 / np.sqrt(D))

# Schraudolph fast-exp constants (bf16 bit pattern via int16):
#   bf16_bits(exp(x)) ~= round(x * 2^7/ln2 + 127*2^7 - C16)
A16 = float(2**7 / np.log(2.0))
C16 = 366393.0 / 65536.0
B16 = float(127 * 2**7) - C16

# neighbor selection (angular-distance graph from the reference model)
SEL = {
    0: [18, 20, 22, 24], 1: [2, 4, 6, 8], 2: [1, 3, 9, 10], 3: [2, 4, 11],
    4: [1, 3, 5, 12], 5: [4, 6, 13], 6: [1, 5, 7, 14], 7: [6, 8, 15],
    8: [1, 7, 9, 16], 9: [2, 8, 17], 10: [2, 11, 17, 18], 11: [3, 10, 12, 19],
    12: [4, 11, 13, 20], 13: [5, 12, 14, 21], 14: [6, 13, 15, 22],
    15: [7, 14, 16, 23], 16: [8, 15, 17, 24], 17: [9, 10, 16, 25],
    18: [0, 10, 19, 25], 19: [11, 18, 20], 20: [0, 12, 19, 21],
    21: [13, 20, 22], 22: [0, 14, 21, 23], 23: [15, 22, 24],
    24: [0, 16, 23, 25], 25: [17, 18, 24],
}
MHA_IDX = [0, 1] + [2] * 8 + [3] * 8 + [4] * 8

N_CORES = 8
# slot types per core: (n_neighbors, q_chunks_of_512)
SLOT_SPEC = [(4, 2), (4, 2), (4, 2), (4, 2), (3, 2), (3, 2), (4, 1)]
N_SLOTS = len(SLOT_SPEC)
KVOFF = np.concatenate([[0], np.cumsum([n for n, _ in SLOT_SPEC])]).astype(int)
KV_ROWS = int(KVOFF[-1])          # 26

# fraction (out of 8) of score tiles evacuated by DVE fast-exp; rest by ACT
DVE_OF_8 = 4

# ---- task assignment: full-view tasks + leftover halves, balanced SPMD
_V4 = [i for i in range(N_VIEWS) if len(SEL[i]) == 4]   # 18 views
_V3 = [i for i in range(N_VIEWS) if len(SEL[i]) == 3]   # 8 views
_T4 = [(b, i) for b in range(2) for i in _V4]           # 36 tasks
_T3 = [(b, i) for b in range(2) for i in _V3]           # 16 tasks
_T4_FULL = _T4[:32]                                     # 4 per core
_T4_HALF = [(b, i, h) for (b, i) in _T4[32:] for h in range(2)]  # 8 halves


def _assign(core):
    """Slot list for a core: entries (b, view, qh0, n_qchunks)."""
    t4 = _T4_FULL[core * 4:(core + 1) * 4]
    t3 = _T3[core * 2:(core + 1) * 2]
    bh, ih, hh = _T4_HALF[core]
    return ([(b, i, 0, 2) for (b, i) in t4] +
            [(b, i, 0, 2) for (b, i) in t3] +
            [(bh, ih, hh, 1)])


ASSIGN = [_assign(c) for c in range(N_CORES)]

_PROGRAM_CACHE = {}


def _build_program(with_bias=False):
    key = ("bias" if with_bias else "fast")
    if key in _PROGRAM_CACHE:
        return _PROGRAM_CACHE[key]

    nc = bacc.Bacc("TRN2", target_bir_lowering=False, debug=False)

    f32, f32r, bf16, i16 = dt.float32, dt.float32r, dt.bfloat16, dt.int16

    xq_d = nc.dram_tensor("xq", [N_SLOTS, C, S], f32, kind="ExternalInput").ap()
    xkv_d = nc.dram_tensor("xkv", [KV_ROWS, C, S], f32, kind="ExternalInput").ap()
    wqkvT_d = nc.dram_tensor("wqkvT", [N_SLOTS, C, 3 * C], f32, kind="ExternalInput").ap()
    woT_d = nc.dram_tensor("woT", [N_SLOTS, C, C], f32, kind="ExternalInput").ap()
    out_d = nc.dram_tensor("out", [N_SLOTS, C, S], f32, kind="ExternalOutput").ap()
    if with_bias:
        bqkv_d = nc.dram_tensor("bqkv", [N_SLOTS, 3 * C, 1], f32, kind="ExternalInput").ap()
        bo_d = nc.dram_tensor("bo", [N_SLOTS, C, 1], f32, kind="ExternalInput").ap()

    # Bresenham-interleaved ACT/DVE assignment for score-tile evacuation
    _ctr = [0]

    def _tile_to_dve():
        k = _ctr[0]
        _ctr[0] += 1
        return (k * DVE_OF_8) // 8 != ((k + 1) * DVE_OF_8) // 8

    from contextlib import ExitStack
    with ExitStack() as stack:
        tc = stack.enter_context(tile.TileContext(nc))
        wp = stack.enter_context(tc.tile_pool(name="wp", bufs=4))
        wop = stack.enter_context(tc.tile_pool(name="wop", bufs=4))
        xqp = stack.enter_context(tc.tile_pool(name="xqp", bufs=4))
        xnp = stack.enter_context(tc.tile_pool(name="xnp", bufs=3))
        qp_pool = stack.enter_context(tc.tile_pool(name="qp", bufs=4))
        kp_pool = stack.enter_context(tc.tile_pool(name="kp", bufs=4))
        vp_pool = stack.enter_context(tc.tile_pool(name="vp", bufs=2))
        esp = stack.enter_context(tc.tile_pool(name="esp", bufs=4))
        avp = stack.enter_context(tc.tile_pool(name="avp", bufs=4))
        otp = stack.enter_context(tc.tile_pool(name="otp", bufs=4))
        recp = stack.enter_context(tc.tile_pool(name="recp", bufs=2))
        rbp = stack.enter_context(tc.tile_pool(name="rbp", bufs=2))
        if with_bias:
            biasp = stack.enter_context(tc.tile_pool(name="biasp", bufs=16))
        avsp = stack.enter_context(tc.tile_pool(name="avsp", bufs=4))
        psc = stack.enter_context(tc.tile_pool(name="psc", bufs=1, space="PSUM"))
        pav_pool = stack.enter_context(tc.tile_pool(name="pav", bufs=1, space="PSUM"))
        ppr = stack.enter_context(tc.tile_pool(name="ppr", bufs=2, space="PSUM"))

        for t in range(N_SLOTS):
            n, qchunks = SLOT_SPEC[t]
            Q = qchunks * 512

            # ---- weights for this slot
            w_sb, wo_sb = [], []
            for ki in range(2):
                w = wp.tile([128, 3 * C], f32r, tag="w")
                nc.sync.dma_start(w, wqkvT_d[t, ki * 128:(ki + 1) * 128, :].bitcast(f32r))
                w_sb.append(w)
                wo = wop.tile([128, C], f32r, tag="wo")
                nc.sync.dma_start(wo, woT_d[t, ki * 128:(ki + 1) * 128, :].bitcast(f32r))
                wo_sb.append(wo)
            if with_bias:
                bq, bk, bv, bo = [], [], [], []
                for mo in range(2):
                    for lst, base in ((bq, 0), (bk, C), (bv, 2 * C)):
                        b_ = biasp.tile([128, 1], f32, tag="bias")
                        nc.sync.dma_start(b_, bqkv_d[t, base + mo * 128: base + (mo + 1) * 128, :])
                        lst.append(b_)
                    b_ = biasp.tile([128, 1], f32, tag="bias")
                    nc.sync.dma_start(b_, bo_d[t, mo * 128:(mo + 1) * 128, :])
                    bo.append(b_)

            def evac(dst, src, bias_ap):
                if with_bias:
                    nc.vector.tensor_scalar_add(dst, src, bias_ap)
                else:
                    nc.vector.tensor_copy(dst, src)

            # ---- q load + projection: qpT [2 mo][128, Q] bf16
            xq_sb = []
            for ki in range(2):
                xq = xqp.tile([128, Q], f32r, tag="xq")
                nc.sync.dma_start(xq, xq_d[t, ki * 128:(ki + 1) * 128, 0:Q].bitcast(f32r))
                xq_sb.append(xq)
            qpT = []
            for mo in range(2):
                q_bf = qp_pool.tile([128, Q], bf16, tag="qpT", name=f"qpT{t}_{mo}")
                for qc in range(qchunks):
                    pq = ppr.tile([128, 512], f32, tag="proj")
                    for ki in range(2):
                        nc.tensor.matmul(pq, w_sb[ki][:, mo * 128:(mo + 1) * 128],
                                         xq_sb[ki][:, qc * 512:(qc + 1) * 512],
                                         start=(ki == 0), stop=(ki == 1))
                    evac(q_bf[:, qc * 512:(qc + 1) * 512], pq,
                         bq[mo] if with_bias else None)
                qpT.append(q_bf)

            # ---- per-neighbor K/V projection (once per view)
            kpT = [kp_pool.tile([128, n * S], bf16, tag="kpT", name=f"kpT{t}_{mo}")
                   for mo in range(2)]
            v_sb = vp_pool.tile([128, n * 8 * 264], bf16, tag="v", name=f"v{t}")
            nc.vector.memset(
                v_sb.rearrange("p (g h e) -> p g h e", h=NH, e=D + 1)[:, :, :, D:D + 1], 1.0)

            for j in range(n):
                xn_sb = []
                for ki in range(2):
                    xn = xnp.tile([128, S], f32r, tag="xn")
                    nc.sync.dma_start(xn, xkv_d[KVOFF[t] + j, ki * 128:(ki + 1) * 128, :].bitcast(f32r))
                    xn_sb.append(xn)
                # K: kpT[mo][:, j*S : (j+1)*S]
                for mo in range(2):
                    for nq in range(2):
                        pk = ppr.tile([128, 512], f32, tag="proj")
                        for ki in range(2):
                            nc.tensor.matmul(pk, w_sb[ki][:, C + mo * 128: C + (mo + 1) * 128],
                                             xn_sb[ki][:, nq * 512:(nq + 1) * 512],
                                             start=(ki == 0), stop=(ki == 1))
                        evac(kpT[mo][:, j * S + nq * 512: j * S + (nq + 1) * 512], pk,
                             bk[mo] if with_bias else None)
                # V (transposed: kv tokens on partitions), 2 chunks per psum bank
                for sp in range(4):
                    pv = ppr.tile([128, 512], f32, tag="proj")
                    for u in range(2):
                        st = sp * 2 + u
                        for ki in range(2):
                            nc.tensor.matmul(pv[:, u * C:(u + 1) * C],
                                             xn_sb[ki][:, st * 128:(st + 1) * 128],
                                             w_sb[ki][:, 2 * C:3 * C],
                                             start=(ki == 0), stop=(ki == 1))
                    g = j * 8 + sp * 2
                    dst = v_sb[:, g * 264:(g + 2) * 264].rearrange(
                        "p (c h e) -> p c h e", c=2, e=D + 1)[:, :, :, 0:D]
                    nc.vector.tensor_copy(
                        dst, pv.rearrange("p (c h d) -> p c h d", c=2, d=D))

            # ---- attention per (q-half, head-group of 4)
            for qh in range(qchunks):
                avnT = [avp.tile([128, 512], f32r, tag="avnT",
                                 name=f"avnT{t}_{qh}_{mo}") for mo in range(2)]
                for pg in range(2):
                    heads = [4 * pg + i for i in range(4)]
                    qtile = qpT[pg][:, qh * 512:(qh + 1) * 512]
                    ktile = kpT[pg]
                    # two head-pairs accumulate in two pav banks
                    pavs = [pav_pool.tile([97, 512], f32, tag=f"av{p}",
                                          name=f"pav_{t}_{qh}_{pg}_{p}")
                            for p in range(2)]
                    for j in range(n):
                        for c in range(8):
                            g = j * 8 + c
                            esc = esp.tile([128, 4 * 512], bf16, tag="es",
                                           name=f"es_{t}_{qh}_{pg}_{j}_{c}")
                            pss4 = [psc.tile([128, 512], f32, tag=f"sc{i}",
                                             name=f"pss{i}")
                                    for i in range(4)]
                            for i in range(4):
                                hp = i * 32
                                nc.tensor.matmul(
                                    pss4[i],
                                    ktile[hp:hp + 32, g * 128:(g + 1) * 128],
                                    qtile[hp:hp + 32, :], start=True, stop=True,
                                    tile_position=(hp, 0))
                            for i in range(4):
                                dst = esc[:, i * 512:(i + 1) * 512]
                                if _tile_to_dve():
                                    nc.vector.tensor_scalar(
                                        dst.bitcast(i16), pss4[i], A16 * ISQ, B16,
                                        op0=AL.mult, op1=AL.add)
                                else:
                                    nc.scalar.activation(dst, pss4[i], AF.Exp, scale=ISQ)
                            st_, sp_ = (j == 0 and c == 0), (j == n - 1 and c == 7)
                            for i in range(4):
                                h = heads[i]
                                pav2 = pavs[i // 2]
                                rows = pav2[0:33, :] if i % 2 == 0 else pav2[64:97, :]
                                cg = 0 if i % 2 == 0 else 64
                                nc.tensor.matmul(
                                    rows, v_sb[:, g * 264 + 33 * h: g * 264 + 33 * h + 33],
                                    esc[:, i * 512:(i + 1) * 512],
                                    start=st_, stop=sp_, tile_position=(0, cg),
                                    skip_group_check=True)
                    # normalize: stage av rows + sums to SBUF (frees pav fast),
                    # then reciprocal + broadcast + multiply
                    for i in range(4):
                        h = heads[i]
                        pav2 = pavs[i // 2]
                        sums_row = pav2[32:33, :] if i % 2 == 0 else pav2[96:97, :]
                        av_rows = pav2[0:32, :] if i % 2 == 0 else pav2[64:96, :]
                        srow = recp.tile([1, 512], f32, tag="rec")
                        nc.vector.tensor_copy(srow, sums_row)
                        avS = avsp.tile([32, 512], f32, tag="avS")
                        nc.vector.tensor_copy(avS, av_rows)
                        rec = recp.tile([1, 512], f32, tag="rec2")
                        nc.vector.reciprocal_approx_fast(rec, srow)
                        rb = rbp.tile([32, 512], f32, tag="rb")
                        nc.gpsimd.partition_broadcast(rb, rec)
                        nc.vector.tensor_mul(avnT[pg][(h % 4) * 32:(h % 4) * 32 + 32, :],
                                             avS, rb)

                if with_bias:
                    for mo in range(2):
                        nc.vector.tensor_scalar_add(avnT[mo], avnT[mo], bv[mo])
                # ---- out-projection for this q-half
                for mo in range(2):
                    po = ppr.tile([128, 512], f32, tag="proj")
                    for ki in range(2):
                        nc.tensor.matmul(po, wo_sb[ki][:, mo * 128:(mo + 1) * 128],
                                         avnT[ki], start=(ki == 0), stop=(ki == 1))
                    oT = otp.tile([128, 512], f32, tag="oT")
                    evac(oT, po, bo[mo] if with_bias else None)
                    nc.sync.dma_start(
                        out_d[t, mo * 128:(mo + 1) * 128, qh * 512:(qh + 1) * 512], oT)

    nc.compile()
    _PROGRAM_CACHE[key] = nc
    return nc


def _prep_inputs(x, w_qkv, b_qkv, w_out, b_out, with_bias):
    x = np.ascontiguousarray(np.asarray(x, dtype=np.float32))
    w_qkv = np.asarray(w_qkv, dtype=np.float32)
    w_out = np.asarray(w_out, dtype=np.float32)
    b_qkv = np.asarray(b_qkv, dtype=np.float32)
    b_out = np.asarray(b_out, dtype=np.float32)

    x2 = x.reshape(2, N_VIEWS, C, S)
    in_maps = []
    for core in range(N_CORES):
        slots = ASSIGN[core]
        xq = np.zeros((N_SLOTS, C, S), np.float32)
        xkv = np.empty((KV_ROWS, C, S), np.float32)
        wqkvT = np.empty((N_SLOTS, C, 3 * C), np.float32)
        woT = np.empty((N_SLOTS, C, C), np.float32)
        if with_bias:
            bqkv = np.empty((N_SLOTS, 3 * C, 1), np.float32)
            bo = np.empty((N_SLOTS, C, 1), np.float32)
        for t, (b, i, qh0, qchunks) in enumerate(slots):
            m = MHA_IDX[i]
            Q = qchunks * 512
            xq[t, :, 0:Q] = x2[b, i][:, qh0 * 512: qh0 * 512 + Q]
            for j, nb in enumerate(SEL[i]):
                xkv[KVOFF[t] + j] = x2[b, nb]
            wqkvT[t] = w_qkv[m].T
            woT[t] = w_out[m].T
            if with_bias:
                bqkv[t, :, 0] = b_qkv[m]
                bo[t, :, 0] = b_out[m]
        im = {"xq": xq, "xkv": xkv, "wqkvT": wqkvT, "woT": woT}
        if with_bias:
            im["bqkv"] = bqkv
            im["bo"] = bo
        in_maps.append(im)
    return in_maps


def _gather_output(results, dtype):
    y = np.empty((2, N_VIEWS, C, S), np.float32)
    for core in range(N_CORES):
        out = results[core]["out"]
        for t, (b, i, qh0, qchunks) in enumerate(ASSIGN[core]):
            Q = qchunks * 512
            y[b, i][:, qh0 * 512: qh0 * 512 + Q] = out[t][:, 0:Q]
    return y.reshape(2 * N_VIEWS, C, 32, 32).astype(dtype, copy=False)


def _run(inputs, trace=False, tmpdir=None):
    with_bias = bool(np.any(inputs["b_qkv"]) or np.any(inputs["b_out"]))
    nc = _build_program(with_bias)
    in_maps = _prep_inputs(**inputs, with_bias=with_bias)
    res = run_bass_kernel_spmd(nc, in_maps, core_ids=list(range(N_CORES)),
                               trace=trace, tmpdir=tmpdir)
    y = _gather_output(res.results, np.asarray(inputs["x"]).dtype)
    return y, res


def kernel(x, w_qkv, b_qkv, w_out, b_out):
    y, _ = _run(dict(x=x, w_qkv=w_qkv, b_qkv=b_qkv, w_out=w_out, b_out=b_out))
    return y
